# revision 1
# baseline (speedup 1.0000x reference)
"""DAG-LSTM Trainium2 kernel.

Problem: 2-layer LSTM scanned over a 48-node DAG, batch 1024, hidden 256.
Sharding: pure data parallelism -- batch split 8 x 128 across NeuronCores,
weights replicated, no cross-device traffic.

Layout: "transposed" (feature-on-partition). Each state h/c is kept as
[H=2x128 partition-chunks, B=128 free], so every LSTM matmul
(gates^T = W^T-chunk.T @ act^T) needs no on-chip transposes. dags is
pre-transposed (and fp16-cast) on the host. DAG nodes that are
simultaneously ready are batched into groups of <=GROUP_MAX so the moving
operand of each matmul is N = u*128, and weights are fp16 (full PE rate +
fast weight load). The c-state path stays fp32; h-path and gate
activations are fp16 (5e-4 rounding).
"""

import sys
import numpy as np

sys.path.insert(0, "/opt/trn_rl_repo")

B, N, IN, H, L, P = 1024, 48, 256, 256, 2, 2
NCORES = 8
BL = B // NCORES          # 128 batch per core
KC = 2                    # K chunks (256 = 2*128)
GROUP_MAX = 4             # nodes per matmul group

# gate-chunk order in the 4H dim: i0 i1 f0 f1 g0 g1 o0 o1
# (= PyTorch row order i f g o; o last so sigmoid(o) can run after the
# DVE products start)
_GATE_PERM = np.r_[0:1024]

_CACHE = {}


def _unit_deps(pred, i, l):
    d = [(int(v) - 1, l) for v in pred[i] if v > 0]
    if l == 1:
        d.append((i, 0))
    return d


def _build_schedule(pred):
    """Schedule units (node, layer) into same-layer groups of <= GROUP_MAX.

    ASAP stage ordering with a per-layer pending buffer so groups are packed
    to GROUP_MAX where legal (no intra-group dependencies)."""
    stage = {}
    for i in range(N):
        d0 = [stage[(int(v) - 1, 0)] for v in pred[i] if v > 0]
        stage[(i, 0)] = (max(d0) + 1) if d0 else 0
        d1 = [stage[(int(v) - 1, 1)] for v in pred[i] if v > 0]
        d1.append(stage[(i, 0)])
        stage[(i, 1)] = max(d1) + 1
    order = sorted(((stage[u], u[1], u[0]) for u in stage))
    buf = {0: [], 1: []}
    groups = []

    def flush(l):
        if buf[l]:
            groups.append((l, buf[l][:]))
            buf[l].clear()

    for (_, l, i) in order:
        # flush any buffer holding a dependency of this unit
        for (dj, dl) in _unit_deps(pred, i, l):
            if dj in buf[dl]:
                flush(dl)
        buf[l].append(i)
        if len(buf[l]) == GROUP_MAX:
            flush(l)
    flush(0)
    flush(1)
    return groups


def _alloc_slots(pred, groups):
    """Per-layer physical slot allocation with liveness-based reuse.

    Returns slot_of[(node, layer)] -> phys slot id, and per-layer counts."""
    gidx = {}
    for g, (l, nodes) in enumerate(groups):
        for i in nodes:
            gidx[(i, l)] = g
    ngroups = len(groups)
    last_read = {}
    for i in range(N):
        last_read[(i, 0)] = gidx[(i, 1)]       # (i,1) x-reads h_l0
        last_read[(i, 1)] = ngroups - 1 if i == N - 1 else -1  # output DMA
    for j in range(N):
        for v in pred[j]:
            if v > 0:
                i = int(v) - 1
                for l in range(L):
                    last_read[(i, l)] = max(last_read[(i, l)], gidx[(j, l)])
    slot_of = {}
    nslots = [0, 0]
    free = {0: [], 1: []}
    frees_at = {}
    for (i, l), lr in last_read.items():
        if lr >= 0:
            frees_at.setdefault(lr, []).append((i, l))
    for g, (l, nodes) in enumerate(groups):
        for i in nodes:
            if free[l]:
                slot_of[(i, l)] = free[l].pop()
            else:
                slot_of[(i, l)] = nslots[l]
                nslots[l] += 1
        for (i, ll) in frees_at.get(g, []):
            free[ll].append(slot_of[(i, ll)])
    return slot_of, nslots


def _prep_weights(w_ih, w_hh, b_ih, b_hh):
    """Host-side weight prep. Returns wx, wh [128, 2, 1024] fp16 and
    bias [8, 128] f32."""
    wx = w_ih[_GATE_PERM]              # [1024, K]
    wh = w_hh[_GATE_PERM] * 0.5        # fold predecessor mean into W_hh
    b = (b_ih + b_hh)[_GATE_PERM]      # [1024]

    def to_t(w):
        kdim = w.shape[1]
        wt = np.ascontiguousarray(w.T)            # [K, 1024]
        wt = wt.reshape(kdim // 128, 128, 1024)   # [kc, kin, 1024]
        return np.ascontiguousarray(
            wt.transpose(1, 0, 2).astype(np.float16))  # [128, kc, 1024]

    return to_t(wx), to_t(wh), np.ascontiguousarray(b.reshape(8, 128))


def _build_program(pred):
    from contextlib import ExitStack
    from concourse import bacc, mybir, tile

    f32 = mybir.dt.float32
    f16 = mybir.dt.float16
    AF = mybir.ActivationFunctionType
    Alu = mybir.AluOpType

    groups = _build_schedule(pred)
    slot_of, nslots = _alloc_slots(pred, groups)

    nc = bacc.Bacc("TRN2", target_bir_lowering=False, debug=False,
                   num_devices=NCORES)

    dags_t = nc.dram_tensor("dags_t", [N, 128, KC, 128], f16,
                            kind="ExternalInput")
    h0_t = nc.dram_tensor("h0_t", [128, L, KC, 128], f16,
                          kind="ExternalInput")
    c0_t = nc.dram_tensor("c0_t", [128, L, KC, 128], f32,
                          kind="ExternalInput")
    w_dram = {}
    for l in range(L):
        w_dram[("x", l)] = nc.dram_tensor(f"wx{l}", [128, KC, 1024], f16,
                                          kind="ExternalInput")
        w_dram[("h", l)] = nc.dram_tensor(f"wh{l}", [128, KC, 1024], f16,
                                          kind="ExternalInput")
    bias_dram = nc.dram_tensor("bias", [128, L, 8], f32, kind="ExternalInput")
    out_t = nc.dram_tensor("out_t", [KC, 128, 128], f32, kind="ExternalOutput")

    with tile.TileContext(nc) as tc, ExitStack() as ctx:
        consts = ctx.enter_context(tc.tile_pool(name="consts", bufs=1))
        ps = ctx.enter_context(tc.tile_pool(name="ps", bufs=8, space="PSUM"))
        gp = ctx.enter_context(tc.tile_pool(name="gp", bufs=3))

        # initial states + bias first (small, on the fast HWDGE queue)
        slot0_h = consts.tile([128, L, KC, 128], f16, tag="slot0h")
        nc.sync.dma_start(out=slot0_h[:], in_=h0_t[:])
        slot0_c = consts.tile([128, L, KC, 128], f32, tag="slot0c")
        nc.sync.dma_start(out=slot0_c[:], in_=c0_t[:])
        bias_sb = consts.tile([128, L, 8], f32, tag="bias")
        nc.sync.dma_start(out=bias_sb[:], in_=bias_dram[:])
        wsb = {}
        for key in [("x", 0), ("h", 0), ("x", 1), ("h", 1)]:
            dt_ = w_dram[key]
            t = consts.tile([128, KC, 1024], f16, tag=f"w{key[0]}{key[1]}",
                            name=f"w{key[0]}{key[1]}")
            nc.gpsimd.dma_start(out=t[:], in_=dt_[:])
            wsb[key] = t
        sl_h = {l: [consts.tile([128, KC, 128], f16, tag=f"sh{l}_{s}",
                                name=f"sh{l}_{s}")
                    for s in range(nslots[l])] for l in range(L)}
        sl_c = {l: [consts.tile([128, KC, 128], f32, tag=f"sc{l}_{s}",
                                name=f"sc{l}_{s}")
                    for s in range(nslots[l])] for l in range(L)}
        outh = consts.tile([128, KC, 128], f32, tag="outh")

        def h_ap(v, l):
            if v == 0:
                return slot0_h[:, l]
            return sl_h[l][slot_of[(v - 1, l)]][:]

        def c_ap(v, l):
            if v == 0:
                return slot0_c[:, l]
            return sl_c[l][slot_of[(v - 1, l)]][:]

        ngather = 0
        for (l, nodes) in groups:
            u = len(nodes)
            xq = gp.tile([128, KC, u, 128], f16, tag="xq", name="xq")
            ubh = gp.tile([128, KC, u, 128], f16, tag="ubh")
            ubc = gp.tile([128, KC, u, 128], f16, tag="ubc")
            sifo = gp.tile([128, 4, u, 128], f16, tag="sifo")
            so = gp.tile([128, u, KC, 128], f16, tag="so")
            gt = gp.tile([128, KC, u, 128], f16, tag="gt")
            vw = gp.tile([128, u, 4, 128], f16, tag="vw")
            th = gp.tile([128, u, KC, 128], f16, tag="th")

            # 1. inputs: x (layer0: DMA; layer1: fast fp16 copy of h_l0)
            for j, i in enumerate(nodes):
                if l == 0:
                    nc.sync.dma_start(out=xq[:, :, j, :], in_=dags_t[i])
                else:
                    nc.vector.tensor_copy(out=xq[:, :, j, :],
                                          in_=sl_h[0][slot_of[(i, 0)]][:])

            # 2. predecessor state sums (mean's 0.5 folded into W_hh / STT)
            for j, i in enumerate(nodes):
                a, b_ = int(pred[i][0]), int(pred[i][1])
                if a == b_:
                    nc.vector.tensor_scalar_mul(ubh[:, :, j, :], h_ap(a, l),
                                                2.0)
                    nc.vector.tensor_scalar_mul(ubc[:, :, j, :], c_ap(a, l),
                                                2.0)
                else:
                    nc.vector.tensor_tensor(out=ubh[:, :, j, :],
                                            in0=h_ap(a, l), in1=h_ap(b_, l),
                                            op=Alu.add)
                    nc.vector.tensor_tensor(out=ubc[:, :, j, :],
                                            in0=c_ap(a, l), in1=c_ap(b_, l),
                                            op=Alu.add)

            # 3. gates: 8 M-chunks x (x,h)x(kc) accumulated matmuls.
            # Layer-1 x operands are read per-unit straight from the h_l0
            # state slots (no staging copy).
            pstiles = []
            for m in range(8):
                pt = ps.tile([128, u * 128], f32, tag="gates", name="gates")
                pstiles.append(pt)
                if l == 0:
                    # x (batched), then h (batched)
                    for k in range(KC):
                        nc.tensor.matmul(
                            out=pt[:],
                            lhsT=wsb[("x", l)][:, k, m * 128:(m + 1) * 128],
                            rhs=xq[:, k].rearrange("p u b -> p (u b)"),
                            start=(k == 0), stop=False)
                    for k in range(KC):
                        nc.tensor.matmul(
                            out=pt[:],
                            lhsT=wsb[("h", l)][:, k, m * 128:(m + 1) * 128],
                            rhs=ubh[:, k].rearrange("p u b -> p (u b)"),
                            start=False, stop=(k == KC - 1))
                else:
                    # h first (ubh is ready earlier than x = h_l0 this stage)
                    for k in range(KC):
                        nc.tensor.matmul(
                            out=pt[:],
                            lhsT=wsb[("h", l)][:, k, m * 128:(m + 1) * 128],
                            rhs=ubh[:, k].rearrange("p u b -> p (u b)"),
                            start=(k == 0), stop=False)
                    for k in range(KC):
                        nc.tensor.matmul(
                            out=pt[:],
                            lhsT=wsb[("x", l)][:, k, m * 128:(m + 1) * 128],
                            rhs=xq[:, k].rearrange("p u b -> p (u b)"),
                            start=False, stop=(k == KC - 1))

            # 4a. sigmoid(i,f) chunks 0..3, tanh(g) chunks 4,5
            for c in range(4):
                nc.scalar.activation(out=sifo[:, c].rearrange("p u b -> p (u b)"),
                                     in_=pstiles[c][:],
                                     func=AF.Sigmoid,
                                     bias=bias_sb[:, l, c:c + 1])
            for cc in range(KC):
                nc.scalar.activation(out=gt[:, cc].rearrange("p u b -> p (u b)"),
                                     in_=pstiles[4 + cc][:],
                                     func=AF.Tanh,
                                     bias=bias_sb[:, l, 4 + cc:5 + cc])

            # 5. w = sigmoid(i) * tanh(g); v = sigmoid(f) * csum
            nc.vector.tensor_tensor(out=vw[:, :, 0:2, :]
                                    .rearrange("p u c b -> p c u b"),
                                    in0=sifo[:, 0:2], in1=gt[:], op=Alu.mult)
            nc.vector.tensor_tensor(out=vw[:, :, 2:4, :]
                                    .rearrange("p u c b -> p c u b"),
                                    in0=sifo[:, 2:4], in1=ubc[:], op=Alu.mult)

            # 4b. sigmoid(o) chunks 6,7 (off the w/v critical path)
            for c in range(6, 8):
                nc.scalar.activation(out=so[:, :, c - 6, :],
                                     in_=pstiles[c][:],
                                     func=AF.Sigmoid,
                                     bias=bias_sb[:, l, c:c + 1])

            # 6..8 per node: c_new = 0.5*v + w; tanh(c); h = sigmoid(o)*tanh(c)
            for j, i in enumerate(nodes):
                cdst = sl_c[l][slot_of[(i, l)]]
                nc.vector.scalar_tensor_tensor(
                    out=cdst[:], in0=vw[:, j, 2:4, :], scalar=0.5,
                    in1=vw[:, j, 0:2, :], op0=Alu.mult, op1=Alu.add)
                nc.scalar.activation(out=th[:, j], in_=cdst[:],
                                     func=AF.Tanh)
                hdst = (outh[:] if (i == N - 1 and l == 1)
                        else sl_h[l][slot_of[(i, l)]][:])
                nc.vector.tensor_tensor(out=hdst, in0=so[:, j],
                                        in1=th[:, j], op=Alu.mult)

        # output: h of last node, top layer: [128, KC, 128] -> [KC, 128, 128]
        nc.sync.dma_start(out=out_t.ap().rearrange("k p b -> p k b"),
                          in_=outh[:])

    nc.compile()
    return nc


def _get_program(pred):
    key = pred.tobytes()
    if key not in _CACHE:
        _CACHE[key] = _build_program(pred)
    return _CACHE[key]


def _prepare(dags, h0, c0, w_ih0, w_hh0, b_ih0, b_hh0,
             w_ih1, w_hh1, b_ih1, b_hh1, pred_idx):
    """Host-side prep: returns (nc, in_maps)."""
    dags = np.asarray(dags, dtype=np.float32)
    h0 = np.asarray(h0, dtype=np.float32)
    c0 = np.asarray(c0, dtype=np.float32)
    pred = np.asarray(pred_idx)

    nc = _get_program(pred)

    wx0, wh0, bias0 = _prep_weights(np.asarray(w_ih0, np.float32),
                                    np.asarray(w_hh0, np.float32),
                                    np.asarray(b_ih0, np.float32),
                                    np.asarray(b_hh0, np.float32))
    wx1, wh1, bias1 = _prep_weights(np.asarray(w_ih1, np.float32),
                                    np.asarray(w_hh1, np.float32),
                                    np.asarray(b_ih1, np.float32),
                                    np.asarray(b_hh1, np.float32))
    bias = np.ascontiguousarray(
        np.stack([bias0.T, bias1.T], axis=1))  # [128, 2, 8]

    in_maps = []
    for c in range(NCORES):
        bs = slice(c * BL, (c + 1) * BL)
        # dags [B, N, IN] -> [N, kin(128), kc, b] fp16
        dt_ = dags[bs].transpose(1, 2, 0).reshape(N, KC, 128, BL)
        dt_ = np.ascontiguousarray(
            dt_.transpose(0, 2, 1, 3).astype(np.float16))
        # h0/c0 [L, B, H] -> [128(p), L, kc, b]
        hh = h0[:, bs, :].transpose(2, 0, 1).reshape(KC, 128, L, BL)
        cc = c0[:, bs, :].transpose(2, 0, 1).reshape(KC, 128, L, BL)
        h0t = np.ascontiguousarray(
            hh.transpose(1, 2, 0, 3).astype(np.float16))  # [128, L, kc, b]
        c0t = np.ascontiguousarray(cc.transpose(1, 2, 0, 3))
        in_maps.append({
            "dags_t": dt_, "h0_t": h0t, "c0_t": c0t,
            "wx0": wx0, "wh0": wh0, "wx1": wx1, "wh1": wh1, "bias": bias,
        })
    return nc, in_maps


def _assemble(res):
    out = np.empty((B, H), np.float32)
    for c in range(NCORES):
        ot = res.results[c]["out_t"]  # [KC, 128, 128] = [kc, p, b]
        out[c * BL:(c + 1) * BL] = ot.reshape(H, BL).T
    return out


def kernel(**inputs):
    from concourse.bass_utils import run_bass_kernel_spmd

    nc, in_maps = _prepare(**inputs)
    res = run_bass_kernel_spmd(nc, in_maps, list(range(NCORES)))
    return _assemble(res)



# revision 8
# speedup vs baseline: 3.6645x; 3.6645x over previous
"""DAG-LSTM Trainium2 kernel.

Problem: 2-layer LSTM scanned over a 48-node DAG, batch 1024, hidden 256.
Sharding: pure data parallelism -- batch split 8 x 128 across NeuronCores,
weights replicated, no cross-device traffic.

Key optimization: the output is only node N-1's top-layer hidden state, so
only the ancestor set of (N-1, layer1) needs to be computed (20 of 96
(node, layer) units for the given DAG).  The live units are scheduled in
ASAP stages (width <= 2 for this DAG); same-stage same-layer units form one
matmul group so the moving operand is N = u*128.

Layout: "transposed" (feature-on-partition).  Each state h/c is a slot in a
per-layer buffer [128 part, slot, KC=2, 128 batch]; every LSTM matmul
(gates^T = W^T-chunk.T @ act^T) needs no on-chip transposes.  dags is
pre-transposed (and fp16-cast) on the host; weights are fp16.  Slots are
allocated in schedule order so a group's units occupy contiguous slots,
letting the pointwise tail (c update, tanh, h product) run group-batched,
and letting layer-1 x-operands be read straight from the layer-0 slots.
"""

import sys
import numpy as np

sys.path.insert(0, "/opt/trn_rl_repo")

B, N, IN, H, L, P = 1024, 48, 256, 256, 2, 2
NCORES = 8
BL = B // NCORES          # 128 batch per core
KC = 2                    # K chunks (256 = 2*128)
GROUP_MAX = 4             # max nodes per matmul group

_CACHE = {}


def _live_units(pred):
    """Ancestors of (N-1, 1): the only units the output depends on."""
    live = set()
    stack = [(N - 1, 1)]
    while stack:
        u = stack.pop()
        if u in live:
            continue
        live.add(u)
        i, l = u
        if l == 1:
            stack.append((i, 0))
        for v in pred[i]:
            if v > 0:
                stack.append((int(v) - 1, l))
    return live


def _build_schedule(pred):
    """Live units -> list of groups (layer, [nodes]), ASAP stages.

    Units in the same (stage, layer) are mutually independent; wide stages
    are split into chunks of GROUP_MAX.  Returns groups in emission order.
    """
    live = _live_units(pred)
    stage = {}
    for i in range(N):
        if (i, 0) in live:
            d = [stage[(int(v) - 1, 0)] for v in pred[i] if v > 0]
            stage[(i, 0)] = (max(d) + 1) if d else 0
        if (i, 1) in live:
            d = [stage[(int(v) - 1, 1)] for v in pred[i] if v > 0]
            d.append(stage[(i, 0)])
            stage[(i, 1)] = max(d) + 1
    bystage = {}
    for (i, l), s in stage.items():
        bystage.setdefault((s, l), []).append(i)
    groups = []
    for (s, l) in sorted(bystage):
        nodes = sorted(bystage[(s, l)])
        for j in range(0, len(nodes), GROUP_MAX):
            groups.append((l, nodes[j:j + GROUP_MAX]))
    return groups


def _prep_weights(w_ih, w_hh, b_ih, b_hh):
    """Host-side weight prep. Returns wx [128, kc_in, 1024] fp16 (kc_in =
    input K chunks), wh [128, KC, 1024] fp16 and bias [8, 128] f32."""
    wx = w_ih                          # [1024, K]
    wh = w_hh * 0.5                    # fold predecessor mean into W_hh
    b = b_ih + b_hh                    # [1024]

    def to_t(w):
        kdim = w.shape[1]
        wt = np.ascontiguousarray(w.T)            # [K, 1024]
        wt = wt.reshape(kdim // 128, 128, 1024)   # [kc, kin, 1024]
        return np.ascontiguousarray(
            wt.transpose(1, 0, 2).astype(np.float16))  # [128, kc, 1024]

    return to_t(wx), to_t(wh), np.ascontiguousarray(b.reshape(8, 128))


def _build_program(pred):
    from contextlib import ExitStack
    from concourse import bacc, mybir, tile

    f32 = mybir.dt.float32
    f16 = mybir.dt.float16
    AF = mybir.ActivationFunctionType
    Alu = mybir.AluOpType

    groups = _build_schedule(pred)
    # slot allocation: slot 0 = initial state, then schedule order per layer
    slot_of = {}
    nslots = [1, 1]
    gidx = {}
    for g, (l, nodes) in enumerate(groups):
        for i in nodes:
            slot_of[(i, l)] = nslots[l]
            nslots[l] += 1
            gidx[(i, l)] = g
    live_l0 = sorted(i for (i, l) in slot_of if l == 0)
    pos_l0 = {i: p for p, i in enumerate(live_l0)}
    nl0 = len(live_l0)

    def src_group(i, l):
        """Group that produced state (i-in-pred-encoding v, l); -1 = init."""
        return gidx.get((i, l), -1)

    nc = bacc.Bacc("TRN2", target_bir_lowering=False, debug=False,
                   num_devices=NCORES)

    dags_t = nc.dram_tensor("dags_t", [nl0, 128, KC, 128], f16,
                            kind="ExternalInput")
    h0_t = nc.dram_tensor("h0_t", [128, L, KC, 128], f16,
                          kind="ExternalInput")
    c0_t = nc.dram_tensor("c0_t", [128, L, KC, 128], f32,
                          kind="ExternalInput")
    w_dram = {}
    for l in range(L):
        w_dram[("x", l)] = nc.dram_tensor(f"wx{l}", [128, KC, 1024], f16,
                                          kind="ExternalInput")
        w_dram[("h", l)] = nc.dram_tensor(f"wh{l}", [128, KC, 1024], f16,
                                          kind="ExternalInput")
    bias_dram = nc.dram_tensor("bias", [128, L, 8], f32, kind="ExternalInput")
    out_t = nc.dram_tensor("out_t", [KC, 128, 128], f32, kind="ExternalOutput")

    with tile.TileContext(nc) as tc, ExitStack() as ctx:
        consts = ctx.enter_context(tc.tile_pool(name="consts", bufs=1))
        ps = ctx.enter_context(tc.tile_pool(name="ps", bufs=8, space="PSUM"))
        gp = ctx.enter_context(tc.tile_pool(name="gp", bufs=3))

        # state buffers: [128, KC, slot, 128]; slot 0 = h0/c0
        bigh = {l: consts.tile([128, KC, nslots[l], 128], f16,
                               tag=f"bigh{l}", name=f"bigh{l}")
                for l in range(L)}
        bigc = {l: consts.tile([128, KC, nslots[l], 128], f32,
                               tag=f"bigc{l}", name=f"bigc{l}")
                for l in range(L)}
        bias_sb = consts.tile([128, L, 8], f32, tag="bias")
        nc.sync.dma_start(out=bias_sb[:], in_=bias_dram[:])
        for l in range(L):
            nc.sync.dma_start(out=bigh[l][:, :, 0, :], in_=h0_t[:, l])
            nc.sync.dma_start(out=bigc[l][:, :, 0, :], in_=c0_t[:, l])
        wsb = {}
        for key in [("x", 0), ("h", 0), ("x", 1), ("h", 1)]:
            t = consts.tile([128, KC, 1024], f16, tag=f"w{key[0]}{key[1]}",
                            name=f"w{key[0]}{key[1]}")
            nc.gpsimd.dma_start(out=t[:], in_=w_dram[key][:])
            wsb[key] = t
        outh = consts.tile([128, KC, 128], f32, tag="outh")

        for g, (l, nodes) in enumerate(groups):
            u = len(nodes)
            s0 = slot_of[(nodes[0], l)]
            xq = gp.tile([128, KC, u, 128], f16, tag="xq", name="xq")
            ubh = gp.tile([128, KC, u, 128], f16, tag="ubh")
            ubc = gp.tile([128, KC, u, 128], f32, tag="ubc")
            sifo = gp.tile([128, 4, u, 128], f16, tag="sifo")
            so = gp.tile([128, KC, u, 128], f16, tag="so")
            gt = gp.tile([128, KC, u, 128], f16, tag="gt")
            vw = gp.tile([128, 4, u, 128], f16, tag="vw")
            th = gp.tile([128, KC, u, 128], f16, tag="th")

            # --- x operand: layer0 from DRAM; layer1 from h_l0 slots
            #     (direct if contiguous, else staged copy)
            x_direct = None
            if l == 0:
                for j, i in enumerate(nodes):
                    nc.sync.dma_start(out=xq[:, :, j, :],
                                      in_=dags_t[pos_l0[i]])
                xdep = -1
            else:
                sx0 = slot_of[(nodes[0], 0)]
                if all(slot_of[(i, 0)] == sx0 + j
                       for j, i in enumerate(nodes)):
                    x_direct = sx0
                else:
                    for j, i in enumerate(nodes):
                        nc.vector.tensor_copy(
                            out=xq[:, :, j, :],
                            in_=bigh[0][:, :, slot_of[(i, 0)], :])
                xdep = max(src_group(i, 0) for i in nodes)

            # --- predecessor state sums (mean folded into W_hh / STT)
            hdep = -1
            for j, i in enumerate(nodes):
                a, b_ = int(pred[i][0]), int(pred[i][1])
                sa = 0 if a == 0 else slot_of[(a - 1, l)]
                sb = 0 if b_ == 0 else slot_of[(b_ - 1, l)]
                for v in (a, b_):
                    if v > 0:
                        hdep = max(hdep, src_group(v - 1, l))
                if sa == sb:
                    nc.vector.tensor_scalar_mul(ubh[:, :, j, :],
                                                bigh[l][:, :, sa, :], 2.0)
                    nc.vector.tensor_scalar_mul(ubc[:, :, j, :],
                                                bigc[l][:, :, sa, :], 2.0)
                else:
                    nc.vector.tensor_tensor(out=ubh[:, :, j, :],
                                            in0=bigh[l][:, :, sa, :],
                                            in1=bigh[l][:, :, sb, :],
                                            op=Alu.add)
                    nc.vector.tensor_tensor(out=ubc[:, :, j, :],
                                            in0=bigc[l][:, :, sa, :],
                                            in1=bigc[l][:, :, sb, :],
                                            op=Alu.add)

            # --- gates: part with the most recent dependency goes second
            def x_rhs(k):
                if x_direct is not None:
                    return (bigh[0][:, k, x_direct:x_direct + u, :]
                            .rearrange("p u b -> p (u b)"))
                return xq[:, k].rearrange("p u b -> p (u b)")

            def h_rhs(k):
                return ubh[:, k].rearrange("p u b -> p (u b)")

            parts = [("x", x_rhs), ("h", h_rhs)]
            if xdep > hdep:
                parts = parts[::-1]
            pstiles = [ps.tile([128, u * 128], f32, tag="gates",
                               name="gates") for _ in range(8)]
            for pi, (op, rhs) in enumerate(parts):
                for m in range(8):
                    for k in range(KC):
                        nc.tensor.matmul(
                            out=pstiles[m][:],
                            lhsT=wsb[(op, l)][:, k, m * 128:(m + 1) * 128],
                            rhs=rhs(k),
                            start=(pi == 0 and k == 0),
                            stop=(pi == 1 and k == KC - 1))

            # --- activations: sigmoid(i,f) chunks 0-3, tanh(g) 4-5
            for c in range(4):
                nc.scalar.activation(out=sifo[:, c].rearrange("p u b -> p (u b)"),
                                     in_=pstiles[c][:],
                                     func=AF.Sigmoid,
                                     bias=bias_sb[:, l, c:c + 1])
            for cc in range(KC):
                nc.scalar.activation(out=gt[:, cc].rearrange("p u b -> p (u b)"),
                                     in_=pstiles[4 + cc][:],
                                     func=AF.Tanh,
                                     bias=bias_sb[:, l, 4 + cc:5 + cc])

            # --- w = sigmoid(i)*tanh(g); v = sigmoid(f)*csum
            nc.vector.tensor_tensor(out=vw[:, 0:2], in0=sifo[:, 0:2],
                                    in1=gt[:], op=Alu.mult)
            nc.vector.tensor_tensor(out=vw[:, 2:4], in0=sifo[:, 2:4],
                                    in1=ubc[:], op=Alu.mult)

            # --- sigmoid(o) chunks 6,7 (off the w/v critical path)
            for c in range(6, 8):
                nc.scalar.activation(out=so[:, c - 6]
                                     .rearrange("p u b -> p (u b)"),
                                     in_=pstiles[c][:],
                                     func=AF.Sigmoid,
                                     bias=bias_sb[:, l, c:c + 1])

            # --- group-batched: c = 0.5*v + w; tanh(c); h = sigmoid(o)*tanh(c)
            cdst = bigc[l][:, :, s0:s0 + u, :]
            nc.vector.scalar_tensor_tensor(
                out=cdst, in0=vw[:, 2:4], scalar=0.5,
                in1=vw[:, 0:2], op0=Alu.mult, op1=Alu.add)
            nc.scalar.activation(out=th[:], in_=cdst, func=AF.Tanh)
            if l == 1 and nodes[-1] == N - 1:
                # final node's h goes to the f32 output staging tile
                if u > 1:
                    nc.vector.tensor_tensor(
                        out=bigh[l][:, :, s0:s0 + u - 1, :],
                        in0=so[:, :, :u - 1, :], in1=th[:, :, :u - 1, :],
                        op=Alu.mult)
                nc.vector.tensor_tensor(
                    out=outh[:], in0=so[:, :, u - 1, :],
                    in1=th[:, :, u - 1, :], op=Alu.mult)
            else:
                nc.vector.tensor_tensor(out=bigh[l][:, :, s0:s0 + u, :],
                                        in0=so[:], in1=th[:], op=Alu.mult)

        # output: h of last node, top layer: [128, KC, 128] -> [KC, 128, 128]
        nc.sync.dma_start(out=out_t.ap().rearrange("k p b -> p k b"),
                          in_=outh[:])

    nc.compile()
    return nc, live_l0


def _get_program(pred):
    key = pred.tobytes()
    if key not in _CACHE:
        _CACHE[key] = _build_program(pred)
    return _CACHE[key]


def _prepare(dags, h0, c0, w_ih0, w_hh0, b_ih0, b_hh0,
             w_ih1, w_hh1, b_ih1, b_hh1, pred_idx):
    """Host-side prep: returns (nc, in_maps)."""
    dags = np.asarray(dags, dtype=np.float32)
    h0 = np.asarray(h0, dtype=np.float32)
    c0 = np.asarray(c0, dtype=np.float32)
    pred = np.asarray(pred_idx)

    nc, live_l0 = _get_program(pred)

    wx0, wh0, bias0 = _prep_weights(np.asarray(w_ih0, np.float32),
                                    np.asarray(w_hh0, np.float32),
                                    np.asarray(b_ih0, np.float32),
                                    np.asarray(b_hh0, np.float32))
    wx1, wh1, bias1 = _prep_weights(np.asarray(w_ih1, np.float32),
                                    np.asarray(w_hh1, np.float32),
                                    np.asarray(b_ih1, np.float32),
                                    np.asarray(b_hh1, np.float32))
    bias = np.ascontiguousarray(
        np.stack([bias0.T, bias1.T], axis=1))  # [128, 2, 8]

    dl = dags[:, live_l0, :]  # [B, nl0, IN]
    nl0 = len(live_l0)
    in_maps = []
    for c in range(NCORES):
        bs = slice(c * BL, (c + 1) * BL)
        # dags [B, nl0, IN] -> [nl0, kin(128), kc, b] fp16
        dt_ = dl[bs].transpose(1, 2, 0).reshape(nl0, KC, 128, BL)
        dt_ = np.ascontiguousarray(
            dt_.transpose(0, 2, 1, 3).astype(np.float16))
        # h0/c0 [L, B, H] -> [128(p), L, kc, b]
        hh = h0[:, bs, :].transpose(2, 0, 1).reshape(KC, 128, L, BL)
        cc = c0[:, bs, :].transpose(2, 0, 1).reshape(KC, 128, L, BL)
        h0t = np.ascontiguousarray(
            hh.transpose(1, 2, 0, 3).astype(np.float16))  # [128, L, kc, b]
        c0t = np.ascontiguousarray(cc.transpose(1, 2, 0, 3))
        in_maps.append({
            "dags_t": dt_, "h0_t": h0t, "c0_t": c0t,
            "wx0": wx0, "wh0": wh0, "wx1": wx1, "wh1": wh1, "bias": bias,
        })
    return nc, in_maps


def _assemble(res):
    out = np.empty((B, H), np.float32)
    for c in range(NCORES):
        ot = res.results[c]["out_t"]  # [KC, 128, 128] = [kc, p, b]
        out[c * BL:(c + 1) * BL] = ot.reshape(H, BL).T
    return out


def kernel(**inputs):
    from concourse.bass_utils import run_bass_kernel_spmd

    nc, in_maps = _prepare(**inputs)
    res = run_bass_kernel_spmd(nc, in_maps, list(range(NCORES)))
    return _assemble(res)
